# revision 2
# baseline (speedup 1.0000x reference)
"""Trainium2 Bass kernel for conv-projected multi-head attention.

Reference computation (per batch item b of 8, one NeuronCore each):
  y   = BN(depthwise3x3(x_b reshaped to [C,32,32]))      # q = k = v = y
  q/k/v = y @ w{q,k,v}^T  (heads: 12 x 32)
  att = softmax((q @ k^T) * sqrt(32))
  out = (att @ v) @ wo^T

v2 design (ACT-exp-bound, PE fully tiled):
 - conv: 9 accumulating diag-matmuls on PE over a zero-padded [34x34] image
 - qT/kT [o, t] f32r via lhsT=w^T; v stored [t, h, d] bf16 (vsb)
 - attention in 8 groups of 3 heads (g in 4, lh in 2 query halves):
     scores: 3 row-tiled (K=32) matmuls -> s4 psum [128, 1536] (3 banks,
       double buffered -> exp never waits on a WAR hazard)
     exp on ACT psum->SBUF bf16, scale=sqrt(32) fused
     AV: 3 col-tiled (M=32) matmuls, lhsT=v_h, each streaming its own E_h,
       accumulated over the 8 t-tiles into one psum bank (rows 32j)
     sums: 3 col-tiled matmuls with lhsT=ones[128,32] -> softmax denoms
       replicated over 32 rows (same row layout as AV output)
     normalize: DVE fast-reciprocal (custom op) + one tensor_tensor mult
       -> attn tiles [96, T] f32r (per head-group-of-3)
 - out projection contracts the 4 attn tiles with woT row-sliced in
   96-chunks; lh=0 half runs overlapped with lh=1 attention.
PSUM: scores 2x3 banks + AV 1 + sums 1 = 8.
"""
import sys

sys.path.insert(0, "/opt/trn_rl_repo")
from contextlib import ExitStack

import numpy as np

B, T, C = 8, 1024, 384
NH, DH = 12, 32
HH = WW = 32
SCALE = float(DH) ** 0.5
BN_EPS = 1e-5
NCORES = 8
NG, HPG = 4, 3  # head groups of 3

_CACHE = {}


def _build(debug=False):
    import concourse.bass as bass
    import concourse.tile as tile
    from concourse import bacc, mybir
    from concourse.masks import make_identity
    from concourse.dve_ops import RECIPROCAL_APPROX_FAST, RECIP_APPROX_FAST_CONSTS

    F32 = mybir.dt.float32
    F32R = mybir.dt.float32r
    BF16 = mybir.dt.bfloat16
    AF = mybir.ActivationFunctionType
    ALU = mybir.AluOpType

    nc = bacc.Bacc("TRN2", target_bir_lowering=False, debug=False)

    xt_d = nc.dram_tensor("xt", [C, T], F32R, kind="ExternalInput").ap()
    w9_d = nc.dram_tensor("w9", [C, 9], F32, kind="ExternalInput").ap()
    bias_d = nc.dram_tensor("bias", [C, 1], F32, kind="ExternalInput").ap()
    wqT_d = nc.dram_tensor("wqT", [C, C], F32R, kind="ExternalInput").ap()
    wkT_d = nc.dram_tensor("wkT", [C, C], F32R, kind="ExternalInput").ap()
    wvT_d = nc.dram_tensor("wvT", [C, C], F32R, kind="ExternalInput").ap()
    woT_d = nc.dram_tensor("woT", [C, C], F32R, kind="ExternalInput").ap()
    ones_d = nc.dram_tensor("ones32", [128, 32], F32, kind="ExternalInput").ap()
    outT_d = nc.dram_tensor("outT", [C, T], F32, kind="ExternalOutput").ap()
    dbg = {}
    if debug:
        dbg["y"] = nc.dram_tensor("dbg_y", [C, T], F32, kind="ExternalOutput").ap()
        dbg["qT"] = nc.dram_tensor("dbg_qT", [C, T], F32, kind="ExternalOutput").ap()
        dbg["vsb"] = nc.dram_tensor(
            "dbg_vsb", [T, C], F32, kind="ExternalOutput"
        ).ap()
        dbg["E0"] = nc.dram_tensor(
            "dbg_E0", [2, T, 3 * 512], F32, kind="ExternalOutput"
        ).ap()
        dbg["ov0"] = nc.dram_tensor(
            "dbg_ov0", [2, 128, 512], F32, kind="ExternalOutput"
        ).ap()
        dbg["attn"] = nc.dram_tensor(
            "dbg_attn", [4, 96, T], F32, kind="ExternalOutput"
        ).ap()

    CT = C // 128  # 3 c-tiles
    TT = T // 128  # 8 t-tiles
    TH = T // 512  # 2 t-halves / l-halves

    with tile.TileContext(nc) as tc, ExitStack() as top:
        # ---- persistent pools ----
        persist = top.enter_context(tc.tile_pool(name="persist", bufs=1))
        copies = top.enter_context(tc.tile_pool(name="copies", bufs=3))

        y_sb = [persist.tile([128, T], F32R, tag=f"y{i}", name=f"y{i}") for i in range(CT)]
        qT_sb = [persist.tile([128, T], F32R, tag=f"q{i}", name=f"q{i}") for i in range(CT)]
        kT_sb = [persist.tile([128, T], F32R, tag=f"k{i}", name=f"k{i}") for i in range(CT)]
        vsb = [persist.tile([128, NH, DH], BF16, tag=f"v{i}", name=f"v{i}") for i in range(TT)]
        attn_sb = [persist.tile([96, T], F32R, tag=f"at{i}", name=f"at{i}") for i in range(NG)]
        ones_bf = persist.tile([128, DH], BF16, tag="ones", name="ones")

        with ExitStack() as ph1:
            convpool = ph1.enter_context(tc.tile_pool(name="convpool", bufs=1))
            conv_ps = ph1.enter_context(
                tc.tile_pool(name="conv_ps", bufs=2, space="PSUM")
            )
            qk_ps = ph1.enter_context(tc.tile_pool(name="qk_ps", bufs=4, space="PSUM"))
            v_ps = ph1.enter_context(tc.tile_pool(name="v_ps", bufs=2, space="PSUM"))

            # preload the exp table set on ACT while conv/DMA run
            warm = convpool.tile([128, 8], F32, tag="warm")
            nc.vector.memset(warm[:], 0.0)
            nc.scalar.activation(warm[:], warm[:], AF.Exp)

            # ---- padded input and diag weights ----
            xt_sb = [convpool.tile([128, T], F32R, tag=f"xt{i}", name=f"xt{i}") for i in range(CT)]
            xp = [convpool.tile([128, 34 * 34], F32R, tag=f"xp{i}", name=f"xp{i}") for i in range(CT)]
            w9_sb = [convpool.tile([128, 9], F32, tag=f"w9{i}", name=f"w9s{i}") for i in range(CT)]
            ident = convpool.tile([128, 128], F32, tag="ident")
            diag = [convpool.tile([128, 9, 128], F32R, tag=f"dg{i}", name=f"dg{i}") for i in range(CT)]

            make_identity(nc, ident[:])
            for i in range(CT):
                nc.sync.dma_start(xt_sb[i][:], xt_d[i * 128 : (i + 1) * 128, :])
                nc.sync.dma_start(w9_sb[i][:], w9_d[i * 128 : (i + 1) * 128, :])
                # gpsimd builds the zero-pad + diag weights; DVE does the
                # strided image copy (keeps the DVE free for psum drains)
                nc.gpsimd.memset(xp[i][:].bitcast(F32), 0.0)
                nc.vector.tensor_copy(
                    xp[i][:].rearrange("p (a b) -> p a b", a=34)[:, 1:33, 1:33],
                    xt_sb[i][:].rearrange("p (a b) -> p a b", a=32),
                )
                for k in range(9):
                    nc.gpsimd.tensor_scalar_mul(
                        diag[i][:, k, :], ident[:], w9_sb[i][:, k : k + 1]
                    )

            # weight / constant DMAs after the conv inputs so xt arrives first
            wT_sb = {}
            for nm, d in (("k", wkT_d), ("q", wqT_d), ("v", wvT_d)):
                tiles = [persist.tile([128, C], F32R, tag=f"w{nm}{i}", name=f"w{nm}{i}") for i in range(CT)]
                for i in range(CT):
                    nc.sync.dma_start(tiles[i][:], d[i * 128 : (i + 1) * 128, :])
                wT_sb[nm] = tiles
            # woT row-sliced in 96-chunks to match the attn tiles
            woT4 = [persist.tile([96, C], F32R, tag=f"wo{i}", name=f"wo{i}") for i in range(NG)]
            for i in range(NG):
                nc.sync.dma_start(woT4[i][:], woT_d[i * 96 : (i + 1) * 96, :])

            bias_sb = [persist.tile([128, 1], F32, tag=f"b{i}", name=f"b{i}") for i in range(CT)]
            for i in range(CT):
                nc.sync.dma_start(bias_sb[i][:], bias_d[i * 128 : (i + 1) * 128, :])
            ones_f32 = convpool.tile([128, DH], F32, tag="ones_f")
            nc.sync.dma_start(ones_f32[:], ones_d)
            nc.vector.tensor_copy(ones_bf[:], ones_f32[:])

            # ---- conv: 9 accumulating diag matmuls per (c-tile, t-half) ----
            for i in range(CT):
                for th in range(TH):
                    yp = conv_ps.tile([128, 512], F32, tag="conv")
                    r0 = th * 16
                    for k in range(9):
                        dy, dx = k // 3 - 1, k % 3 - 1
                        off = (r0 + 1 + dy) * 34 + (1 + dx)
                        rhs = bass.AP(
                            tensor=xp[i].tensor,
                            offset=xp[i].offset + off,
                            ap=[list(p) for p in xp[i].ap[:1]] + [[34, 16], [1, 32]],
                        )
                        nc.tensor.matmul(
                            yp[:].rearrange("p (a b) -> p a b", a=16),
                            diag[i][:, k, :],
                            rhs,
                            start=(k == 0),
                            stop=(k == 8),
                        )
                    nc.vector.tensor_scalar_add(
                        y_sb[i][:, th * 512 : (th + 1) * 512],
                        yp[:],
                        bias_sb[i][:],
                    )
            if debug:
                for i in range(CT):
                    nc.sync.dma_start(
                        dbg["y"][i * 128 : (i + 1) * 128, :], y_sb[i][:].bitcast(F32)
                    )

            # ---- q/k projections: qT[o, t] ----
            # kT fully first, then qT lh=0, then qT lh=1 so attention(lh=0)
            # can start as early as possible
            def proj_qk(nm, dst, ths):
                for ot in range(CT):
                    for th in ths:
                        pp = qk_ps.tile([128, 512], F32, tag="qk")
                        for kt in range(CT):
                            nc.tensor.matmul(
                                pp[:],
                                wT_sb[nm][kt][:, ot * 128 : (ot + 1) * 128],
                                y_sb[kt][:, th * 512 : (th + 1) * 512],
                                start=(kt == 0),
                                stop=(kt == CT - 1),
                            )
                        nc.vector.tensor_copy(
                            dst[ot][:, th * 512 : (th + 1) * 512], pp[:]
                        )

            proj_qk("k", kT_sb, range(TH))
            proj_qk("q", qT_sb, [0])

            # ---- v projection: vsb[t, h, d] bf16 ----
            for tt in range(TT):
                vp = v_ps.tile([128, C], F32, tag="v")
                for kt in range(CT):
                    nc.tensor.matmul(
                        vp[:],
                        y_sb[kt][:, tt * 128 : (tt + 1) * 128],
                        wT_sb["v"][kt][:],
                        start=(kt == 0),
                        stop=(kt == CT - 1),
                    )
                nc.vector.tensor_copy(
                    vsb[tt][:], vp[:].rearrange("p (h d) -> p h d", h=NH)
                )

            proj_qk("q", qT_sb, [1])
            if debug:
                for i in range(CT):
                    nc.sync.dma_start(
                        dbg["qT"][i * 128 : (i + 1) * 128, :], qT_sb[i][:].bitcast(F32)
                    )
                for tt in range(TT):
                    nc.sync.dma_start(
                        dbg["vsb"][tt * 128 : (tt + 1) * 128, :],
                        vsb[tt][:].rearrange("p h d -> p (h d)"),
                    )

        # ---- attention + out projection ----
        with ExitStack() as ph2:
            s_ps = ph2.enter_context(tc.tile_pool(name="s_ps", bufs=2, space="PSUM"))
            ov_ps = ph2.enter_context(tc.tile_pool(name="ov_ps", bufs=1, space="PSUM"))
            sm_ps = ph2.enter_context(tc.tile_pool(name="sm_ps", bufs=1, space="PSUM"))
            epool = ph2.enter_context(tc.tile_pool(name="epool", bufs=10))
            rpool = ph2.enter_context(tc.tile_pool(name="rpool", bufs=2))

            def out_proj(th):
                for ot in range(CT):
                    op = sm_ps.tile([128, 512], F32, tag="sm")
                    for kt in range(NG):
                        nc.tensor.matmul(
                            op[:],
                            woT4[kt][:, ot * 128 : (ot + 1) * 128],
                            attn_sb[kt][:, th * 512 : (th + 1) * 512],
                            start=(kt == 0),
                            stop=(kt == NG - 1),
                        )
                    oc = copies.tile([128, 512], F32, tag="oc")
                    nc.vector.tensor_copy(oc[:], op[:])
                    nc.sync.dma_start(
                        outT_d[ot * 128 : (ot + 1) * 128, th * 512 : (th + 1) * 512],
                        oc[:],
                    )

            for lh in range(TH):
                for g in range(NG):
                    heads = [HPG * g + j for j in range(HPG)]
                    E = [None] * TT
                    ov = ov_ps.tile([128, 512], F32, tag="ov")
                    sm = sm_ps.tile([128, 512], F32, tag="sm")

                    def av_sums(tt):
                        for j, h in enumerate(heads):
                            nc.tensor.matmul(
                                ov[32 * j : 32 * (j + 1), :],
                                vsb[tt][:, h, :],
                                E[tt][:, 512 * j : 512 * (j + 1)],
                                start=(tt == 0),
                                stop=(tt == TT - 1),
                                tile_position=(0, 32 * j),
                            )
                        for j, h in enumerate(heads):
                            nc.tensor.matmul(
                                sm[32 * j : 32 * (j + 1), :],
                                ones_bf[:],
                                E[tt][:, 512 * j : 512 * (j + 1)],
                                start=(tt == 0),
                                stop=(tt == TT - 1),
                                tile_position=(0, 32 * j),
                            )

                    for tt in range(TT):
                        s4 = s_ps.tile([128, HPG * 512], F32, tag="s4")
                        for j, h in enumerate(heads):
                            ct, r = h // 4, 32 * (h % 4)
                            nc.tensor.matmul(
                                s4[:, 512 * j : 512 * (j + 1)],
                                kT_sb[ct][r : r + 32, tt * 128 : (tt + 1) * 128],
                                qT_sb[ct][r : r + 32, lh * 512 : (lh + 1) * 512],
                                start=True,
                                stop=True,
                                tile_position=(r, 0),
                            )
                        e = epool.tile([128, HPG * 512], BF16, tag="E")
                        nc.scalar.activation(e[:], s4[:], AF.Exp, scale=SCALE)
                        E[tt] = e
                        if debug and g == 0 and lh < 2:
                            nc.sync.dma_start(
                                dbg["E0"][lh, tt * 128 : (tt + 1) * 128, :],
                                e[:],
                            )
                        if tt >= 1:
                            av_sums(tt - 1)
                    av_sums(TT - 1)

                    rr = rpool.tile([96, 512], F32, tag="rr")
                    nc.vector._custom_dve(
                        RECIPROCAL_APPROX_FAST,
                        out=rr[:],
                        in0=sm[0:96, :],
                        s0=RECIP_APPROX_FAST_CONSTS["s0"],
                        s1=RECIP_APPROX_FAST_CONSTS["s1"],
                        imm2=RECIP_APPROX_FAST_CONSTS["imm2"],
                    )
                    if debug and g == 0 and lh == 0:
                        nc.sync.dma_start(dbg["ov0"][0], ov[:].bitcast(F32))
                        nc.sync.dma_start(dbg["ov0"][1], sm[:].bitcast(F32))
                    nc.vector.tensor_tensor(
                        attn_sb[g][:, lh * 512 : (lh + 1) * 512],
                        ov[0:96, :],
                        rr[:],
                        ALU.mult,
                    )
                # out-projection for this query half overlaps the next half
                out_proj(lh)
            if debug:
                for g in range(NG):
                    nc.sync.dma_start(
                        dbg["attn"][g], attn_sb[g][:].bitcast(F32)
                    )

    nc.compile()
    return nc


def _prep_inputs(x, conv_w, bn_gamma, bn_beta, bn_mean, bn_var, wq, wk, wv, wo):
    f32 = np.float32
    inv = (bn_gamma / np.sqrt(bn_var + BN_EPS)).astype(f32)
    w9 = (conv_w.reshape(C, 9) * inv[:, None]).astype(f32)
    bias = (bn_beta - bn_mean * inv).astype(f32).reshape(C, 1)
    wqT = np.ascontiguousarray(np.asarray(wq, f32).T)
    wkT = np.ascontiguousarray(np.asarray(wk, f32).T)
    wvT = np.ascontiguousarray(np.asarray(wv, f32).T)
    woT = np.ascontiguousarray(np.asarray(wo, f32).T)
    ones32 = np.ones((128, 32), f32)
    maps = []
    for b in range(B):
        maps.append(
            {
                "xt": np.ascontiguousarray(np.asarray(x[b], f32).T),
                "w9": w9,
                "bias": bias,
                "wqT": wqT,
                "wkT": wkT,
                "wvT": wvT,
                "woT": woT,
                "ones32": ones32,
            }
        )
    return maps


def kernel(x, conv_w, bn_gamma, bn_beta, bn_mean, bn_var, wq, wk, wv, wo, h, w,
           **kw):
    assert int(h) == HH and int(w) == WW
    from concourse.bass_utils import run_bass_kernel_spmd

    if "nc" not in _CACHE:
        _CACHE["nc"] = _build()
    nc = _CACHE["nc"]
    maps = _prep_inputs(
        x, conv_w, bn_gamma, bn_beta, bn_mean, bn_var, wq, wk, wv, wo
    )
    res = run_bass_kernel_spmd(nc, maps, list(range(NCORES)))
    out = np.stack([res.results[b]["outT"].T for b in range(B)])
    return out.astype(np.float32)


# revision 5
# speedup vs baseline: 1.0903x; 1.0903x over previous
"""Trainium2 Bass kernel for conv-projected multi-head attention.

Reference computation (per batch item b of 8, one NeuronCore each):
  y   = BN(depthwise3x3(x_b reshaped to [C,32,32]))      # q = k = v = y
  q/k/v = y @ w{q,k,v}^T  (heads: 12 x 32)
  att = softmax((q @ k^T) * sqrt(32))
  out = (att @ v) @ wo^T

v3 design (ACT-exp-bound, PE fully tiled):
 - conv: 9 accumulating diag-matmuls on PE over a zero-padded [34x34] image
 - qT/kT [o, t] f32r via lhsT=w^T; v stored [t, h, d] bf16 (vsb)
 - scores/exp in 8 groups of 3 heads (sg in 4, lh in 2 query halves):
     3 row-tiled (K=32) matmuls -> s4 psum [128, 1536] (3 banks, double
     buffered so the ACT exp never waits on a WAR hazard); exp on ACT
     psum->SBUF bf16 with scale=sqrt(32) fused.
 - AV/sums in 6 groups of 4 heads (a in 3, lh in 2), all 4 col strips:
     AV: 4 col-tiled (M=32) matmuls, lhsT=v_h, each streaming its own E_h,
       accumulated over the 8 t-tiles into one psum bank (rows 32j)
     sums: 4 col-tiled matmuls with lhsT=ones[128,32] -> softmax denoms
       replicated over 32 rows (same row layout as AV output)
     normalize: DVE fast-reciprocal (custom op) + one [128,512]
       tensor_tensor mult -> attn c-tiles [128, T] f32r
 - out projection contracts the 3 attn c-tiles with woT; the lh=0 half
   runs overlapped with lh=1 attention.
PSUM: scores 2x3 banks + AV 1 + sums/outproj 1 = 8.
"""
import sys

sys.path.insert(0, "/opt/trn_rl_repo")
from contextlib import ExitStack

import numpy as np

B, T, C = 8, 1024, 384
NH, DH = 12, 32
HH = WW = 32
SCALE = float(DH) ** 0.5
BN_EPS = 1e-5
NCORES = 8
SG, HPS = 4, 3  # score groups: 4 groups of 3 heads
AG, HPA = 3, 4  # AV groups: 3 groups of 4 heads

_CACHE = {}


def _build(debug=False):
    import concourse.bass as bass
    import concourse.tile as tile
    from concourse import bacc, mybir
    from concourse.masks import make_identity
    from concourse.dve_ops import RECIPROCAL_APPROX_FAST, RECIP_APPROX_FAST_CONSTS

    F32 = mybir.dt.float32
    F32R = mybir.dt.float32r
    BF16 = mybir.dt.bfloat16
    AF = mybir.ActivationFunctionType
    ALU = mybir.AluOpType

    nc = bacc.Bacc("TRN2", target_bir_lowering=False, debug=False)

    xt_d = nc.dram_tensor("xt", [C, T], F32R, kind="ExternalInput").ap()
    w9_d = nc.dram_tensor("w9", [C, 9], F32, kind="ExternalInput").ap()
    bias_d = nc.dram_tensor("bias", [C, 1], F32, kind="ExternalInput").ap()
    wqT_d = nc.dram_tensor("wqT", [C, C], F32R, kind="ExternalInput").ap()
    wkT_d = nc.dram_tensor("wkT", [C, C], F32R, kind="ExternalInput").ap()
    wvT_d = nc.dram_tensor("wvT", [C, C], F32R, kind="ExternalInput").ap()
    woT_d = nc.dram_tensor("woT", [C, C], F32R, kind="ExternalInput").ap()
    ones_d = nc.dram_tensor("ones32", [128, 32], F32, kind="ExternalInput").ap()
    outT_d = nc.dram_tensor("outT", [C, T], F32, kind="ExternalOutput").ap()
    dbg = {}
    if debug:
        dbg["y"] = nc.dram_tensor("dbg_y", [C, T], F32, kind="ExternalOutput").ap()
        dbg["qT"] = nc.dram_tensor("dbg_qT", [C, T], F32, kind="ExternalOutput").ap()
        dbg["attn"] = nc.dram_tensor(
            "dbg_attn", [C, T], F32, kind="ExternalOutput"
        ).ap()

    CT = C // 128  # 3 c-tiles
    TT = T // 128  # 8 t-tiles
    TH = T // 512  # 2 t-halves / l-halves

    with tile.TileContext(nc) as tc, ExitStack() as top:
        # ---- persistent pools ----
        persist = top.enter_context(tc.tile_pool(name="persist", bufs=1))
        copies = top.enter_context(tc.tile_pool(name="copies", bufs=3))

        y_sb = [persist.tile([128, T], F32R, tag=f"y{i}", name=f"y{i}") for i in range(CT)]
        qT_sb = [persist.tile([128, T], F32R, tag=f"q{i}", name=f"q{i}") for i in range(CT)]
        kT_sb = [persist.tile([128, T], F32R, tag=f"k{i}", name=f"k{i}") for i in range(CT)]
        vsb = [persist.tile([128, NH, DH], BF16, tag=f"v{i}", name=f"v{i}") for i in range(TT)]
        attn_sb = [persist.tile([128, T], F32R, tag=f"at{i}", name=f"at{i}") for i in range(CT)]
        ones_bf = persist.tile([128, DH], BF16, tag="ones", name="ones")

        with ExitStack() as ph1:
            convpool = ph1.enter_context(tc.tile_pool(name="convpool", bufs=1))
            conv_ps = ph1.enter_context(
                tc.tile_pool(name="conv_ps", bufs=2, space="PSUM")
            )
            qk_ps = ph1.enter_context(tc.tile_pool(name="qk_ps", bufs=4, space="PSUM"))
            v_ps = ph1.enter_context(tc.tile_pool(name="v_ps", bufs=2, space="PSUM"))

            # preload the exp table set on ACT while conv/DMA run
            warm = convpool.tile([128, 8], F32, tag="warm")
            nc.vector.memset(warm[:], 0.0)
            nc.scalar.activation(warm[:], warm[:], AF.Exp)

            # ---- padded input and diag weights ----
            xt_sb = [convpool.tile([128, T], F32R, tag=f"xt{i}", name=f"xt{i}") for i in range(CT)]
            xp = [convpool.tile([128, 34 * 34], F32R, tag=f"xp{i}", name=f"xp{i}") for i in range(CT)]
            w9_sb = [convpool.tile([128, 9], F32, tag=f"w9{i}", name=f"w9s{i}") for i in range(CT)]
            ident = convpool.tile([128, 128], F32, tag="ident")
            diag = [convpool.tile([128, 9, 128], F32R, tag=f"dg{i}", name=f"dg{i}") for i in range(CT)]

            make_identity(nc, ident[:])
            for i in range(CT):
                nc.sync.dma_start(xt_sb[i][:], xt_d[i * 128 : (i + 1) * 128, :])
                nc.sync.dma_start(w9_sb[i][:], w9_d[i * 128 : (i + 1) * 128, :])
                nc.vector.memset(xp[i][:].bitcast(F32), 0.0)
                nc.vector.tensor_copy(
                    xp[i][:].rearrange("p (a b) -> p a b", a=34)[:, 1:33, 1:33],
                    xt_sb[i][:].rearrange("p (a b) -> p a b", a=32),
                )
                for k in range(9):
                    nc.vector.tensor_scalar_mul(
                        diag[i][:, k, :], ident[:], w9_sb[i][:, k : k + 1]
                    )

            # weight / constant DMAs after the conv inputs so xt arrives first
            wT_sb = {}
            for nm, d in (("k", wkT_d), ("q", wqT_d), ("v", wvT_d), ("o", woT_d)):
                tiles = [persist.tile([128, C], F32R, tag=f"w{nm}{i}", name=f"w{nm}{i}") for i in range(CT)]
                for i in range(CT):
                    nc.sync.dma_start(tiles[i][:], d[i * 128 : (i + 1) * 128, :])
                wT_sb[nm] = tiles

            bias_sb = [persist.tile([128, 1], F32, tag=f"b{i}", name=f"b{i}") for i in range(CT)]
            for i in range(CT):
                nc.sync.dma_start(bias_sb[i][:], bias_d[i * 128 : (i + 1) * 128, :])
            ones_f32 = convpool.tile([128, DH], F32, tag="ones_f")
            nc.sync.dma_start(ones_f32[:], ones_d)
            nc.vector.tensor_copy(ones_bf[:], ones_f32[:])

            # ---- conv: 9 accumulating diag matmuls per (c-tile, t-half) ----
            for i in range(CT):
                for th in range(TH):
                    yp = conv_ps.tile([128, 512], F32, tag="conv")
                    r0 = th * 16
                    for k in range(9):
                        dy, dx = k // 3 - 1, k % 3 - 1
                        off = (r0 + 1 + dy) * 34 + (1 + dx)
                        rhs = bass.AP(
                            tensor=xp[i].tensor,
                            offset=xp[i].offset + off,
                            ap=[list(p) for p in xp[i].ap[:1]] + [[34, 16], [1, 32]],
                        )
                        nc.tensor.matmul(
                            yp[:].rearrange("p (a b) -> p a b", a=16),
                            diag[i][:, k, :],
                            rhs,
                            start=(k == 0),
                            stop=(k == 8),
                        )
                    nc.vector.tensor_scalar_add(
                        y_sb[i][:, th * 512 : (th + 1) * 512],
                        yp[:],
                        bias_sb[i][:],
                    )
            if debug:
                for i in range(CT):
                    nc.sync.dma_start(
                        dbg["y"][i * 128 : (i + 1) * 128, :], y_sb[i][:].bitcast(F32)
                    )

            # ---- q/k projections: qT[o, t] ----
            # kT fully first, then qT lh=0, then qT lh=1 so attention(lh=0)
            # can start as early as possible
            def proj_qk(nm, dst, ths):
                for ot in range(CT):
                    for th in ths:
                        pp = qk_ps.tile([128, 512], F32, tag="qk")
                        for kt in range(CT):
                            nc.tensor.matmul(
                                pp[:],
                                wT_sb[nm][kt][:, ot * 128 : (ot + 1) * 128],
                                y_sb[kt][:, th * 512 : (th + 1) * 512],
                                start=(kt == 0),
                                stop=(kt == CT - 1),
                            )
                        nc.vector.tensor_copy(
                            dst[ot][:, th * 512 : (th + 1) * 512], pp[:]
                        )

            proj_qk("k", kT_sb, range(TH))
            proj_qk("q", qT_sb, [0])

            # ---- v projection: vsb[t, h, d] bf16 ----
            for tt in range(TT):
                vp = v_ps.tile([128, C], F32, tag="v")
                for kt in range(CT):
                    nc.tensor.matmul(
                        vp[:],
                        y_sb[kt][:, tt * 128 : (tt + 1) * 128],
                        wT_sb["v"][kt][:],
                        start=(kt == 0),
                        stop=(kt == CT - 1),
                    )
                nc.vector.tensor_copy(
                    vsb[tt][:], vp[:].rearrange("p (h d) -> p h d", h=NH)
                )

            proj_qk("q", qT_sb, [1])
            if debug:
                for i in range(CT):
                    nc.sync.dma_start(
                        dbg["qT"][i * 128 : (i + 1) * 128, :], qT_sb[i][:].bitcast(F32)
                    )

        # ---- attention + out projection ----
        with ExitStack() as ph2:
            s_ps = ph2.enter_context(tc.tile_pool(name="s_ps", bufs=2, space="PSUM"))
            ov_ps = ph2.enter_context(tc.tile_pool(name="ov_ps", bufs=1, space="PSUM"))
            sm_ps = ph2.enter_context(tc.tile_pool(name="sm_ps", bufs=1, space="PSUM"))
            epool = ph2.enter_context(tc.tile_pool(name="epool", bufs=20))
            rpool = ph2.enter_context(tc.tile_pool(name="rpool", bufs=2))

            def out_proj(th):
                for ot in range(CT):
                    op = sm_ps.tile([128, 512], F32, tag="sm")
                    for kt in range(CT):
                        nc.tensor.matmul(
                            op[:],
                            wT_sb["o"][kt][:, ot * 128 : (ot + 1) * 128],
                            attn_sb[kt][:, th * 512 : (th + 1) * 512],
                            start=(kt == 0),
                            stop=(kt == CT - 1),
                        )
                    oc = copies.tile([128, 512], F32, tag="oc")
                    nc.vector.tensor_copy(oc[:], op[:])
                    nc.sync.dma_start(
                        outT_d[ot * 128 : (ot + 1) * 128, th * 512 : (th + 1) * 512],
                        oc[:],
                    )

            # E[g][tt] tiles of the current lh; AV group a consumes score
            # groups a and a+1
            for lh in range(TH):
                E = [[None] * TT for _ in range(SG)]
                av_state = {}

                def eslice(h, tt):
                    g, j = h // HPS, h % HPS
                    return E[g][tt][:, 512 * j : 512 * (j + 1)]

                def av_sums(a, tt):
                    ov, sm = av_state[a]
                    for j in range(HPA):
                        h = HPA * a + j
                        nc.tensor.matmul(
                            ov[32 * j : 32 * (j + 1), :],
                            vsb[tt][:, h, :],
                            eslice(h, tt),
                            start=(tt == 0),
                            stop=(tt == TT - 1),
                            tile_position=(0, 32 * j),
                        )
                    for j in range(HPA):
                        h = HPA * a + j
                        nc.tensor.matmul(
                            sm[32 * j : 32 * (j + 1), :],
                            ones_bf[:],
                            eslice(h, tt),
                            start=(tt == 0),
                            stop=(tt == TT - 1),
                            tile_position=(0, 32 * j),
                        )

                def normalize(a):
                    ov, sm = av_state[a]
                    rr = rpool.tile([128, 512], F32, tag="rr")
                    nc.vector._custom_dve(
                        RECIPROCAL_APPROX_FAST,
                        out=rr[:],
                        in0=sm[:],
                        s0=RECIP_APPROX_FAST_CONSTS["s0"],
                        s1=RECIP_APPROX_FAST_CONSTS["s1"],
                        imm2=RECIP_APPROX_FAST_CONSTS["imm2"],
                    )
                    nc.vector.tensor_tensor(
                        attn_sb[a][:, lh * 512 : (lh + 1) * 512],
                        ov[:],
                        rr[:],
                        ALU.mult,
                    )

                for g in range(SG):
                    heads = [HPS * g + j for j in range(HPS)]
                    a = g - 1  # AV group drafting behind this score group
                    if a >= 0:
                        ovt = ov_ps.tile([128, 512], F32, tag="ov", name="ovt")
                        smt = sm_ps.tile([128, 512], F32, tag="sm", name="smt")
                        av_state[a] = (ovt, smt)
                    for tt in range(TT):
                        s4 = s_ps.tile([128, HPS * 512], F32, tag="s4")
                        for j, h in enumerate(heads):
                            ct, r = h // 4, 32 * (h % 4)
                            nc.tensor.matmul(
                                s4[:, 512 * j : 512 * (j + 1)],
                                kT_sb[ct][r : r + 32, tt * 128 : (tt + 1) * 128],
                                qT_sb[ct][r : r + 32, lh * 512 : (lh + 1) * 512],
                                start=True,
                                stop=True,
                                tile_position=(r, 0),
                            )
                        e = epool.tile([128, HPS * 512], BF16, tag="E")
                        nc.scalar.activation(e[:], s4[:], AF.Exp, scale=SCALE)
                        E[g][tt] = e
                        if a >= 0 and tt >= 1:
                            av_sums(a, tt - 1)
                    if a >= 0:
                        av_sums(a, TT - 1)
                        normalize(a)
                # out-projection for this query half overlaps the next half
                out_proj(lh)
            if debug:
                for i in range(CT):
                    nc.sync.dma_start(
                        dbg["attn"][i * 128 : (i + 1) * 128, :],
                        attn_sb[i][:].bitcast(F32),
                    )

    nc.compile()
    return nc


def _prep_inputs(x, conv_w, bn_gamma, bn_beta, bn_mean, bn_var, wq, wk, wv, wo):
    f32 = np.float32
    inv = (bn_gamma / np.sqrt(bn_var + BN_EPS)).astype(f32)
    w9 = (conv_w.reshape(C, 9) * inv[:, None]).astype(f32)
    bias = (bn_beta - bn_mean * inv).astype(f32).reshape(C, 1)
    wqT = np.ascontiguousarray(np.asarray(wq, f32).T)
    wkT = np.ascontiguousarray(np.asarray(wk, f32).T)
    wvT = np.ascontiguousarray(np.asarray(wv, f32).T)
    woT = np.ascontiguousarray(np.asarray(wo, f32).T)
    ones32 = np.ones((128, 32), f32)
    maps = []
    for b in range(B):
        maps.append(
            {
                "xt": np.ascontiguousarray(np.asarray(x[b], f32).T),
                "w9": w9,
                "bias": bias,
                "wqT": wqT,
                "wkT": wkT,
                "wvT": wvT,
                "woT": woT,
                "ones32": ones32,
            }
        )
    return maps


def kernel(x, conv_w, bn_gamma, bn_beta, bn_mean, bn_var, wq, wk, wv, wo, h, w,
           **kw):
    assert int(h) == HH and int(w) == WW
    from concourse.bass_utils import run_bass_kernel_spmd

    if "nc" not in _CACHE:
        _CACHE["nc"] = _build()
    nc = _CACHE["nc"]
    maps = _prep_inputs(
        x, conv_w, bn_gamma, bn_beta, bn_mean, bn_var, wq, wk, wv, wo
    )
    res = run_bass_kernel_spmd(nc, maps, list(range(NCORES)))
    out = np.stack([res.results[b]["outT"].T for b in range(B)])
    return out.astype(np.float32)


# revision 6
# speedup vs baseline: 1.4979x; 1.3739x over previous
"""Trainium2 Bass kernel for conv-projected multi-head attention.

Reference computation (per batch item b of 8, one NeuronCore each):
  y   = BN(depthwise3x3(x_b reshaped to [C,32,32]))      # q = k = v = y
  q/k/v = y @ w{q,k,v}^T  (heads: 12 x 32)
  att = softmax((q @ k^T) * sqrt(32))
  out = (att @ v) @ wo^T

v4 design (ACT-exp-bound; bf16 everywhere so FWL hides all 128-col
LDWEIGHTS; batched DMAs):
 - conv: 9 accumulating diag-matmuls on PE over a zero-padded [34x34] image
 - qT/kT [o, t] bf16 via lhsT=w^T; v stored [t, h, d] bf16 (vsb)
 - scores/exp in 8 groups of 3 heads (sg in 4, lh in 2 query halves):
     3 row-tiled (K=32) matmuls -> s4 psum [128, 1536] (3 banks, double
     buffered so the ACT exp never waits on a WAR hazard); exp on ACT
     psum->SBUF bf16 with scale=sqrt(32) fused.
 - AV/sums in 6 groups of 4 heads (a in 3, lh in 2), all 4 col strips:
     AV: 4 col-tiled (M=32) matmuls, lhsT=v_h, each streaming its own E_h,
       accumulated over the 8 t-tiles into one psum bank (rows 32j)
     sums: 4 col-tiled matmuls with lhsT=ones[128,32] -> softmax denoms
       replicated over 32 rows (same row layout as AV output)
     normalize: DVE fast-reciprocal (custom op) + one [128,512]
       tensor_tensor mult -> attn c-tiles [128, T] bf16
 - out projection contracts the 3 attn c-tiles with woT; the lh=0 half
   runs overlapped with lh=1 attention.
PSUM: scores 2x3 banks + AV 1 + sums/outproj 1 = 8.
"""
import sys

sys.path.insert(0, "/opt/trn_rl_repo")
from contextlib import ExitStack

import numpy as np

B, T, C = 8, 1024, 384
NH, DH = 12, 32
HH = WW = 32
SCALE = float(DH) ** 0.5
BN_EPS = 1e-5
NCORES = 8
SG, HPS = 4, 3  # score groups: 4 groups of 3 heads
AG, HPA = 3, 4  # AV groups: 3 groups of 4 heads

_CACHE = {}


def _build(debug=False):
    import concourse.bass as bass
    import concourse.tile as tile
    from concourse import bacc, mybir
    from concourse.masks import make_identity
    from concourse.dve_ops import RECIPROCAL_APPROX_FAST, RECIP_APPROX_FAST_CONSTS

    F32 = mybir.dt.float32
    BF16 = mybir.dt.bfloat16
    AF = mybir.ActivationFunctionType
    ALU = mybir.AluOpType

    nc = bacc.Bacc("TRN2", target_bir_lowering=False, debug=False)

    xt_d = nc.dram_tensor("xt", [C, T], BF16, kind="ExternalInput").ap()
    w9_d = nc.dram_tensor("w9", [C, 9], F32, kind="ExternalInput").ap()
    bias_d = nc.dram_tensor("bias", [C, 1], F32, kind="ExternalInput").ap()
    wqT_d = nc.dram_tensor("wqT", [C, C], BF16, kind="ExternalInput").ap()
    wkT_d = nc.dram_tensor("wkT", [C, C], BF16, kind="ExternalInput").ap()
    wvT_d = nc.dram_tensor("wvT", [C, C], BF16, kind="ExternalInput").ap()
    woT_d = nc.dram_tensor("woT", [C, C], BF16, kind="ExternalInput").ap()
    ones_d = nc.dram_tensor("ones32", [128, 32], F32, kind="ExternalInput").ap()
    outT_d = nc.dram_tensor("outT", [C, T], F32, kind="ExternalOutput").ap()
    dbg = {}
    if debug:
        dbg["y"] = nc.dram_tensor("dbg_y", [C, T], F32, kind="ExternalOutput").ap()
        dbg["qT"] = nc.dram_tensor("dbg_qT", [C, T], F32, kind="ExternalOutput").ap()
        dbg["attn"] = nc.dram_tensor(
            "dbg_attn", [C, T], F32, kind="ExternalOutput"
        ).ap()

    CT = C // 128  # 3 c-tiles
    TT = T // 128  # 8 t-tiles
    TH = T // 512  # 2 t-halves / l-halves

    with tile.TileContext(nc) as tc, ExitStack() as top:
        # ---- persistent pools ----
        persist = top.enter_context(tc.tile_pool(name="persist", bufs=1))
        copies = top.enter_context(tc.tile_pool(name="copies", bufs=3))

        y_sb = [persist.tile([128, T], BF16, tag=f"y{i}", name=f"y{i}") for i in range(CT)]
        qT_sb = [persist.tile([128, T], BF16, tag=f"q{i}", name=f"q{i}") for i in range(CT)]
        kT_sb = [persist.tile([128, T], BF16, tag=f"k{i}", name=f"k{i}") for i in range(CT)]
        vsb = [persist.tile([128, NH, DH], BF16, tag=f"v{i}", name=f"v{i}") for i in range(TT)]
        attn_sb = [persist.tile([128, T], BF16, tag=f"at{i}", name=f"at{i}") for i in range(CT)]
        ones_bf = persist.tile([128, DH], BF16, tag="ones", name="ones")
        # batched weight tiles: one DMA per matrix
        wsb = {}
        for nm in ("k", "q", "v", "o"):
            wsb[nm] = persist.tile([128, CT, C], BF16, tag=f"w{nm}", name=f"w{nm}")

        with ExitStack() as ph1:
            convpool = ph1.enter_context(tc.tile_pool(name="convpool", bufs=1))
            conv_ps = ph1.enter_context(
                tc.tile_pool(name="conv_ps", bufs=2, space="PSUM")
            )
            qk_ps = ph1.enter_context(tc.tile_pool(name="qk_ps", bufs=4, space="PSUM"))
            v_ps = ph1.enter_context(tc.tile_pool(name="v_ps", bufs=2, space="PSUM"))

            # ---- all input DMAs first, batched, in consumption order ----
            xtb = convpool.tile([128, CT, T], BF16, tag="xtb", name="xtb")
            nc.sync.dma_start(xtb[:], xt_d.rearrange("(i p) t -> p i t", p=128))
            w9_sb = convpool.tile([128, CT, 9], F32, tag="w9b", name="w9b")
            nc.sync.dma_start(w9_sb[:], w9_d.rearrange("(i p) n -> p i n", p=128))
            bias_sb = convpool.tile([128, CT], F32, tag="biasb", name="biasb")
            nc.sync.dma_start(bias_sb[:], bias_d.rearrange("(i p) n -> p (i n)", p=128))
            for nm, d in (("k", wkT_d), ("q", wqT_d), ("v", wvT_d), ("o", woT_d)):
                nc.sync.dma_start(wsb[nm][:], d.rearrange("(i p) n -> p i n", p=128))
            ones_f32 = convpool.tile([128, DH], F32, tag="ones_f")
            nc.sync.dma_start(ones_f32[:], ones_d)

            # preload the exp table set on ACT while conv/DMA run
            warm = convpool.tile([128, 8], F32, tag="warm")
            nc.vector.memset(warm[:], 0.0)
            nc.scalar.activation(warm[:], warm[:], AF.Exp)
            nc.vector.tensor_copy(ones_bf[:], ones_f32[:])

            # ---- padded input and diag weights ----
            xp = [convpool.tile([128, 34 * 34], BF16, tag=f"xp{i}", name=f"xp{i}") for i in range(CT)]
            ident = convpool.tile([128, 128], F32, tag="ident")
            diag = [convpool.tile([128, 9, 128], BF16, tag=f"dg{i}", name=f"dg{i}") for i in range(CT)]

            make_identity(nc, ident[:])
            for i in range(CT):
                nc.vector.memset(xp[i][:].bitcast(F32), 0.0)
                nc.vector.tensor_copy(
                    xp[i][:].rearrange("p (a b) -> p a b", a=34)[:, 1:33, 1:33],
                    xtb[:, i, :].rearrange("p (a b) -> p a b", a=32),
                )
                for k in range(9):
                    nc.vector.tensor_scalar_mul(
                        diag[i][:, k, :], ident[:], w9_sb[:, i, k : k + 1]
                    )

            # ---- conv: 9 accumulating diag matmuls per (c-tile, t-half) ----
            for i in range(CT):
                for th in range(TH):
                    yp = conv_ps.tile([128, 512], F32, tag="conv")
                    r0 = th * 16
                    for k in range(9):
                        dy, dx = k // 3 - 1, k % 3 - 1
                        off = (r0 + 1 + dy) * 34 + (1 + dx)
                        rhs = bass.AP(
                            tensor=xp[i].tensor,
                            offset=xp[i].offset + off,
                            ap=[list(p) for p in xp[i].ap[:1]] + [[34, 16], [1, 32]],
                        )
                        nc.tensor.matmul(
                            yp[:].rearrange("p (a b) -> p a b", a=16),
                            diag[i][:, k, :],
                            rhs,
                            start=(k == 0),
                            stop=(k == 8),
                        )
                    nc.vector.tensor_scalar_add(
                        y_sb[i][:, th * 512 : (th + 1) * 512],
                        yp[:],
                        bias_sb[:, i : i + 1],
                    )
            if debug:
                for i in range(CT):
                    nc.sync.dma_start(
                        dbg["y"][i * 128 : (i + 1) * 128, :], y_sb[i][:]
                    )

            # ---- q/k projections: qT[o, t] ----
            # kT fully first, then qT lh=0, then qT lh=1 so attention(lh=0)
            # can start as early as possible
            def proj_qk(nm, dst, ths):
                for ot in range(CT):
                    for th in ths:
                        pp = qk_ps.tile([128, 512], F32, tag="qk")
                        for kt in range(CT):
                            nc.tensor.matmul(
                                pp[:],
                                wsb[nm][:, kt, ot * 128 : (ot + 1) * 128],
                                y_sb[kt][:, th * 512 : (th + 1) * 512],
                                start=(kt == 0),
                                stop=(kt == CT - 1),
                            )
                        nc.vector.tensor_copy(
                            dst[ot][:, th * 512 : (th + 1) * 512], pp[:]
                        )

            proj_qk("k", kT_sb, range(TH))
            proj_qk("q", qT_sb, [0])

            # ---- v projection: vsb[t, h, d] bf16 ----
            for tt in range(TT):
                vp = v_ps.tile([128, C], F32, tag="v")
                for kt in range(CT):
                    nc.tensor.matmul(
                        vp[:],
                        y_sb[kt][:, tt * 128 : (tt + 1) * 128],
                        wsb["v"][:, kt, :],
                        start=(kt == 0),
                        stop=(kt == CT - 1),
                    )
                nc.vector.tensor_copy(
                    vsb[tt][:], vp[:].rearrange("p (h d) -> p h d", h=NH)
                )

            proj_qk("q", qT_sb, [1])
            if debug:
                for i in range(CT):
                    nc.sync.dma_start(
                        dbg["qT"][i * 128 : (i + 1) * 128, :], qT_sb[i][:]
                    )

        # ---- attention + out projection ----
        with ExitStack() as ph2:
            s_ps = ph2.enter_context(tc.tile_pool(name="s_ps", bufs=2, space="PSUM"))
            ov_ps = ph2.enter_context(tc.tile_pool(name="ov_ps", bufs=1, space="PSUM"))
            sm_ps = ph2.enter_context(tc.tile_pool(name="sm_ps", bufs=1, space="PSUM"))
            epool = ph2.enter_context(tc.tile_pool(name="epool", bufs=20))
            rpool = ph2.enter_context(tc.tile_pool(name="rpool", bufs=2))

            def out_proj(th):
                for ot in range(CT):
                    op = sm_ps.tile([128, 512], F32, tag="sm")
                    for kt in range(CT):
                        nc.tensor.matmul(
                            op[:],
                            wsb["o"][:, kt, ot * 128 : (ot + 1) * 128],
                            attn_sb[kt][:, th * 512 : (th + 1) * 512],
                            start=(kt == 0),
                            stop=(kt == CT - 1),
                        )
                    oc = copies.tile([128, 512], F32, tag="oc")
                    nc.vector.tensor_copy(oc[:], op[:])
                    nc.sync.dma_start(
                        outT_d[ot * 128 : (ot + 1) * 128, th * 512 : (th + 1) * 512],
                        oc[:],
                    )

            # E[g][tt] tiles of the current lh; AV group a consumes score
            # groups a and a+1
            for lh in range(TH):
                E = [[None] * TT for _ in range(SG)]
                av_state = {}

                def eslice(h, tt):
                    g, j = h // HPS, h % HPS
                    return E[g][tt][:, 512 * j : 512 * (j + 1)]

                def av_sums(a, tt):
                    ov, sm = av_state[a]
                    for j in range(HPA):
                        h = HPA * a + j
                        nc.tensor.matmul(
                            ov[32 * j : 32 * (j + 1), :],
                            vsb[tt][:, h, :],
                            eslice(h, tt),
                            start=(tt == 0),
                            stop=(tt == TT - 1),
                            tile_position=(0, 32 * j),
                        )
                    for j in range(HPA):
                        h = HPA * a + j
                        nc.tensor.matmul(
                            sm[32 * j : 32 * (j + 1), :],
                            ones_bf[:],
                            eslice(h, tt),
                            start=(tt == 0),
                            stop=(tt == TT - 1),
                            tile_position=(0, 32 * j),
                        )

                def normalize(a):
                    ov, sm = av_state[a]
                    rr = rpool.tile([128, 512], F32, tag="rr")
                    nc.vector._custom_dve(
                        RECIPROCAL_APPROX_FAST,
                        out=rr[:],
                        in0=sm[:],
                        s0=RECIP_APPROX_FAST_CONSTS["s0"],
                        s1=RECIP_APPROX_FAST_CONSTS["s1"],
                        imm2=RECIP_APPROX_FAST_CONSTS["imm2"],
                    )
                    nc.vector.tensor_tensor(
                        attn_sb[a][:, lh * 512 : (lh + 1) * 512],
                        ov[:],
                        rr[:],
                        ALU.mult,
                    )

                for g in range(SG):
                    heads = [HPS * g + j for j in range(HPS)]
                    a = g - 1  # AV group drafting behind this score group
                    if a >= 0:
                        ovt = ov_ps.tile([128, 512], F32, tag="ov", name="ovt")
                        smt = sm_ps.tile([128, 512], F32, tag="sm", name="smt")
                        av_state[a] = (ovt, smt)
                    for tt in range(TT):
                        s4 = s_ps.tile([128, HPS * 512], F32, tag="s4")
                        for j, h in enumerate(heads):
                            ct, r = h // 4, 32 * (h % 4)
                            nc.tensor.matmul(
                                s4[:, 512 * j : 512 * (j + 1)],
                                kT_sb[ct][r : r + 32, tt * 128 : (tt + 1) * 128],
                                qT_sb[ct][r : r + 32, lh * 512 : (lh + 1) * 512],
                                start=True,
                                stop=True,
                                tile_position=(r, 0),
                            )
                        e = epool.tile([128, HPS * 512], BF16, tag="E")
                        nc.scalar.activation(e[:], s4[:], AF.Exp, scale=SCALE)
                        E[g][tt] = e
                        if a >= 0 and tt >= 1:
                            av_sums(a, tt - 1)
                    if a >= 0:
                        av_sums(a, TT - 1)
                        normalize(a)
                # out-projection for this query half overlaps the next half
                out_proj(lh)
            if debug:
                for i in range(CT):
                    nc.sync.dma_start(
                        dbg["attn"][i * 128 : (i + 1) * 128, :],
                        attn_sb[i][:],
                    )

    nc.compile()
    return nc


def _prep_inputs(x, conv_w, bn_gamma, bn_beta, bn_mean, bn_var, wq, wk, wv, wo):
    import ml_dtypes

    f32 = np.float32
    bf16 = ml_dtypes.bfloat16
    inv = (bn_gamma / np.sqrt(bn_var + BN_EPS)).astype(f32)
    w9 = (conv_w.reshape(C, 9) * inv[:, None]).astype(f32)
    bias = (bn_beta - bn_mean * inv).astype(f32).reshape(C, 1)
    wqT = np.ascontiguousarray(np.asarray(wq, f32).T).astype(bf16)
    wkT = np.ascontiguousarray(np.asarray(wk, f32).T).astype(bf16)
    wvT = np.ascontiguousarray(np.asarray(wv, f32).T).astype(bf16)
    woT = np.ascontiguousarray(np.asarray(wo, f32).T).astype(bf16)
    ones32 = np.ones((128, 32), f32)
    maps = []
    for b in range(B):
        maps.append(
            {
                "xt": np.ascontiguousarray(np.asarray(x[b], f32).T).astype(bf16),
                "w9": w9,
                "bias": bias,
                "wqT": wqT,
                "wkT": wkT,
                "wvT": wvT,
                "woT": woT,
                "ones32": ones32,
            }
        )
    return maps


def kernel(x, conv_w, bn_gamma, bn_beta, bn_mean, bn_var, wq, wk, wv, wo, h, w,
           **kw):
    assert int(h) == HH and int(w) == WW
    from concourse.bass_utils import run_bass_kernel_spmd

    if "nc" not in _CACHE:
        _CACHE["nc"] = _build()
    nc = _CACHE["nc"]
    maps = _prep_inputs(
        x, conv_w, bn_gamma, bn_beta, bn_mean, bn_var, wq, wk, wv, wo
    )
    res = run_bass_kernel_spmd(nc, maps, list(range(NCORES)))
    out = np.stack([res.results[b]["outT"].T for b in range(B)])
    return out.astype(np.float32)


# revision 7
# speedup vs baseline: 1.6174x; 1.0798x over previous
"""Trainium2 Bass kernel for conv-projected multi-head attention.

Reference computation (per batch item b of 8, one NeuronCore each):
  y   = BN(depthwise3x3(x_b reshaped to [C,32,32]))      # q = k = v = y
  q/k/v = y @ w{q,k,v}^T  (heads: 12 x 32)
  att = softmax((q @ k^T) * sqrt(32))
  out = (att @ v) @ wo^T

v5 design (ACT-exp-bound; bf16 everywhere so FWL hides all 128-col
LDWEIGHTS; batched DMAs; projections interleaved INTO the attention
stream so the first exp fires right after the conv):
 - conv: 9 accumulating diag-matmuls on PE over a zero-padded [34x34] image
 - qT/kT [o, t] bf16 via lhsT=w^T; v stored [t, h, d] bf16 (vsb)
 - scores/exp in 8 groups of 3 heads (sg in 4, lh in 2 query halves):
     3 row-tiled (K=32) matmuls -> s4 psum [128, 1536] (3 banks, double
     buffered so the ACT exp never waits on a WAR hazard); exp on ACT
     psum->SBUF bf16 with scale=sqrt(32) fused.
 - AV/sums in 6 groups of 4 heads, all 4 col strips:
     AV: 4 col-tiled (M=32) matmuls, lhsT=v_h, each streaming its own E_h,
       accumulated over the 8 t-tiles into one psum bank (rows 32j)
     sums: 4 col-tiled matmuls with lhsT=ones[128,32] -> softmax denoms
     normalize: DVE fast-reciprocal + one [128,512] tensor_tensor mult
       -> attn c-tiles [128, T] bf16
   AV lags 2 score-groups in the lh=0 half (so the shared psum tag is
   free for the interleaved projections) and 1 group in the lh=1 half.
 - out projection: lh=0 half is emitted inside lh=1's first score group
   (a window with no AV work); lh=1 at the end.
PSUM: tag s4 2x3 banks + tag ps512 2x1 banks (conv/proj/ov/sm/outproj
ring) = 8 banks.
"""
import sys

sys.path.insert(0, "/opt/trn_rl_repo")
from contextlib import ExitStack

import numpy as np

B, T, C = 8, 1024, 384
NH, DH = 12, 32
HH = WW = 32
SCALE = float(DH) ** 0.5
BN_EPS = 1e-5
NCORES = 8
SG, HPS = 4, 3  # score groups: 4 groups of 3 heads
AG, HPA = 3, 4  # AV groups: 3 groups of 4 heads

_CACHE = {}


def _build(debug=False):
    import concourse.bass as bass
    import concourse.tile as tile
    from concourse import bacc, mybir
    from concourse.masks import make_identity
    from concourse.dve_ops import RECIPROCAL_APPROX_FAST, RECIP_APPROX_FAST_CONSTS

    F32 = mybir.dt.float32
    BF16 = mybir.dt.bfloat16
    AF = mybir.ActivationFunctionType
    ALU = mybir.AluOpType

    nc = bacc.Bacc("TRN2", target_bir_lowering=False, debug=False)

    xt_d = nc.dram_tensor("xt", [C, T], BF16, kind="ExternalInput").ap()
    w9_d = nc.dram_tensor("w9", [C, 9], F32, kind="ExternalInput").ap()
    bias_d = nc.dram_tensor("bias", [C, 1], F32, kind="ExternalInput").ap()
    wqT_d = nc.dram_tensor("wqT", [C, C], BF16, kind="ExternalInput").ap()
    wkT_d = nc.dram_tensor("wkT", [C, C], BF16, kind="ExternalInput").ap()
    wvT_d = nc.dram_tensor("wvT", [C, C], BF16, kind="ExternalInput").ap()
    woT_d = nc.dram_tensor("woT", [C, C], BF16, kind="ExternalInput").ap()
    ones_d = nc.dram_tensor("ones32", [128, 32], F32, kind="ExternalInput").ap()
    outT_d = nc.dram_tensor("outT", [C, T], F32, kind="ExternalOutput").ap()

    CT = C // 128  # 3 c-tiles
    TT = T // 128  # 8 t-tiles
    TH = T // 512  # 2 t-halves / l-halves

    with tile.TileContext(nc) as tc, ExitStack() as top:
        persist = top.enter_context(tc.tile_pool(name="persist", bufs=1))
        copies = top.enter_context(tc.tile_pool(name="copies", bufs=3))
        psum = top.enter_context(tc.tile_pool(name="psum", bufs=2, space="PSUM"))
        epool = top.enter_context(tc.tile_pool(name="epool", bufs=32))
        rpool = top.enter_context(tc.tile_pool(name="rpool", bufs=2))
        setup = top.enter_context(tc.tile_pool(name="setup", bufs=1))

        y_sb = [persist.tile([128, T], BF16, tag=f"y{i}", name=f"y{i}") for i in range(CT)]
        qT_sb = [persist.tile([128, T], BF16, tag=f"q{i}", name=f"q{i}") for i in range(CT)]
        kT_sb = [persist.tile([128, T], BF16, tag=f"k{i}", name=f"k{i}") for i in range(CT)]
        vsb = [persist.tile([128, NH, DH], BF16, tag=f"v{i}", name=f"v{i}") for i in range(TT)]
        attn_sb = [persist.tile([128, T], BF16, tag=f"at{i}", name=f"at{i}") for i in range(CT)]
        ones_bf = persist.tile([128, DH], BF16, tag="ones", name="ones")
        wsb = {}
        for nm in ("k", "q", "v", "o"):
            wsb[nm] = persist.tile([128, CT, C], BF16, tag=f"w{nm}", name=f"w{nm}")

        def ps512(name):
            return psum.tile([128, 512], F32, tag="ps512", name=name)

        # ---- all input DMAs first, batched, in consumption order ----
        xtb = setup.tile([128, CT, T], BF16, tag="xtb", name="xtb")
        nc.sync.dma_start(xtb[:], xt_d.rearrange("(i p) t -> p i t", p=128))
        w9_sb = setup.tile([128, CT, 9], F32, tag="w9b", name="w9b")
        nc.sync.dma_start(w9_sb[:], w9_d.rearrange("(i p) n -> p i n", p=128))
        bias_sb = setup.tile([128, CT], F32, tag="biasb", name="biasb")
        nc.sync.dma_start(bias_sb[:], bias_d.rearrange("(i p) n -> p (i n)", p=128))
        for nm, d in (("k", wkT_d), ("q", wqT_d), ("v", wvT_d), ("o", woT_d)):
            nc.sync.dma_start(wsb[nm][:], d.rearrange("(i p) n -> p i n", p=128))
        ones_f32 = setup.tile([128, DH], F32, tag="ones_f")
        nc.sync.dma_start(ones_f32[:], ones_d)

        # preload the exp table set on ACT while conv/DMA run
        warm = setup.tile([128, 8], F32, tag="warm")
        nc.vector.memset(warm[:], 0.0)
        nc.scalar.activation(warm[:], warm[:], AF.Exp)
        nc.vector.tensor_copy(ones_bf[:], ones_f32[:])

        # ---- padded input and diag weights ----
        xp = [setup.tile([128, 34 * 34], BF16, tag=f"xp{i}", name=f"xp{i}") for i in range(CT)]
        ident = setup.tile([128, 128], F32, tag="ident")
        diag = [setup.tile([128, 9, 128], BF16, tag=f"dg{i}", name=f"dg{i}") for i in range(CT)]

        make_identity(nc, ident[:])
        for i in range(CT):
            nc.vector.memset(xp[i][:].bitcast(F32), 0.0)
            nc.vector.tensor_copy(
                xp[i][:].rearrange("p (a b) -> p a b", a=34)[:, 1:33, 1:33],
                xtb[:, i, :].rearrange("p (a b) -> p a b", a=32),
            )
            for k in range(9):
                nc.vector.tensor_scalar_mul(
                    diag[i][:, k, :], ident[:], w9_sb[:, i, k : k + 1]
                )

        # ---- conv: 9 accumulating diag matmuls per (c-tile, t-half) ----
        for i in range(CT):
            for th in range(TH):
                yp = ps512("yp")
                r0 = th * 16
                for k in range(9):
                    dy, dx = k // 3 - 1, k % 3 - 1
                    off = (r0 + 1 + dy) * 34 + (1 + dx)
                    rhs = bass.AP(
                        tensor=xp[i].tensor,
                        offset=xp[i].offset + off,
                        ap=[list(p) for p in xp[i].ap[:1]] + [[34, 16], [1, 32]],
                    )
                    nc.tensor.matmul(
                        yp[:].rearrange("p (a b) -> p a b", a=16),
                        diag[i][:, k, :],
                        rhs,
                        start=(k == 0),
                        stop=(k == 8),
                    )
                nc.vector.tensor_scalar_add(
                    y_sb[i][:, th * 512 : (th + 1) * 512],
                    yp[:],
                    bias_sb[:, i : i + 1],
                )

        # ---- projection emitters (interleaved into the attention stream) --
        def proj_qk(nm, dst, ot, ths):
            for th in ths:
                pp = ps512("pp")
                for kt in range(CT):
                    nc.tensor.matmul(
                        pp[:],
                        wsb[nm][:, kt, ot * 128 : (ot + 1) * 128],
                        y_sb[kt][:, th * 512 : (th + 1) * 512],
                        start=(kt == 0),
                        stop=(kt == CT - 1),
                    )
                nc.vector.tensor_copy(dst[ot][:, th * 512 : (th + 1) * 512], pp[:])

        def proj_v():
            for tt in range(TT):
                vp = ps512("vp")
                for kt in range(CT):
                    nc.tensor.matmul(
                        vp[:, 0:C],
                        y_sb[kt][:, tt * 128 : (tt + 1) * 128],
                        wsb["v"][:, kt, :],
                        start=(kt == 0),
                        stop=(kt == CT - 1),
                    )
                nc.vector.tensor_copy(
                    vsb[tt][:], vp[:, 0:C].rearrange("p (h d) -> p h d", h=NH)
                )

        def out_proj(th):
            for ot in range(CT):
                op = ps512("op")
                for kt in range(CT):
                    nc.tensor.matmul(
                        op[:],
                        wsb["o"][:, kt, ot * 128 : (ot + 1) * 128],
                        attn_sb[kt][:, th * 512 : (th + 1) * 512],
                        start=(kt == 0),
                        stop=(kt == CT - 1),
                    )
                oc = copies.tile([128, 512], F32, tag="oc")
                nc.vector.tensor_copy(oc[:], op[:])
                nc.sync.dma_start(
                    outT_d[ot * 128 : (ot + 1) * 128, th * 512 : (th + 1) * 512],
                    oc[:],
                )

        # ---- attention emitters ----
        state = {"E": None, "av": {}, "lh": 0}

        def scores_group(g, av_plan=None):
            """Emit score group g (3 heads) for current lh; av_plan is an
            optional (a, lagged) pair to interleave AV/sums matmuls."""
            lh = state["lh"]
            heads = [HPS * g + j for j in range(HPS)]
            for tt in range(TT):
                s4 = psum.tile([128, HPS * 512], F32, tag="s4", name="s4")
                for j, h in enumerate(heads):
                    ct, r = h // 4, 32 * (h % 4)
                    nc.tensor.matmul(
                        s4[:, 512 * j : 512 * (j + 1)],
                        kT_sb[ct][r : r + 32, tt * 128 : (tt + 1) * 128],
                        qT_sb[ct][r : r + 32, lh * 512 : (lh + 1) * 512],
                        start=True,
                        stop=True,
                        tile_position=(r, 0),
                    )
                e = epool.tile([128, HPS * 512], BF16, tag="E", name="e")
                nc.scalar.activation(e[:], s4[:], AF.Exp, scale=SCALE)
                state["E"][g][tt] = e
                if av_plan is not None:
                    a, lagged = av_plan
                    if lagged:
                        if tt >= 1:
                            av_sums(a, tt - 1)
                    else:
                        av_sums(a, tt)
            if av_plan is not None:
                a, lagged = av_plan
                if lagged:
                    av_sums(a, TT - 1)
                normalize(a)

        def eslice(h, tt):
            g, j = h // HPS, h % HPS
            return state["E"][g][tt][:, 512 * j : 512 * (j + 1)]

        def av_alloc(a):
            ovt = ps512("ovt")
            smt = ps512("smt")
            state["av"][a] = (ovt, smt)

        def av_sums(a, tt):
            ov, sm = state["av"][a]
            for j in range(HPA):
                h = HPA * a + j
                nc.tensor.matmul(
                    ov[32 * j : 32 * (j + 1), :],
                    vsb[tt][:, h, :],
                    eslice(h, tt),
                    start=(tt == 0),
                    stop=(tt == TT - 1),
                    tile_position=(0, 32 * j),
                )
            for j in range(HPA):
                h = HPA * a + j
                nc.tensor.matmul(
                    sm[32 * j : 32 * (j + 1), :],
                    ones_bf[:],
                    eslice(h, tt),
                    start=(tt == 0),
                    stop=(tt == TT - 1),
                    tile_position=(0, 32 * j),
                )

        def normalize(a):
            lh = state["lh"]
            ov, sm = state["av"][a]
            rr = rpool.tile([128, 512], F32, tag="rr", name="rr")
            nc.vector._custom_dve(
                RECIPROCAL_APPROX_FAST,
                out=rr[:],
                in0=sm[:],
                s0=RECIP_APPROX_FAST_CONSTS["s0"],
                s1=RECIP_APPROX_FAST_CONSTS["s1"],
                imm2=RECIP_APPROX_FAST_CONSTS["imm2"],
            )
            nc.vector.tensor_tensor(
                attn_sb[a][:, lh * 512 : (lh + 1) * 512],
                ov[:],
                rr[:],
                ALU.mult,
            )

        # ---- the schedule ----
        # lh = 0: projections interleaved between score groups, AV lagging
        # two groups so the shared ps512 ring is projection-free by then.
        state["lh"], state["E"] = 0, [[None] * TT for _ in range(SG)]
        proj_qk("k", kT_sb, 0, range(TH))
        proj_qk("q", qT_sb, 0, [0])
        scores_group(0)
        proj_qk("k", kT_sb, 1, range(TH))
        proj_qk("q", qT_sb, 1, [0])
        scores_group(1)
        proj_qk("k", kT_sb, 2, range(TH))
        proj_qk("q", qT_sb, 2, [0])
        proj_v()
        for ot in range(CT):
            proj_qk("q", qT_sb, ot, [1])
        av_alloc(0)
        scores_group(2, av_plan=(0, False))
        av_alloc(1)
        scores_group(3, av_plan=(1, False))
        av_alloc(2)
        for tt in range(TT):
            av_sums(2, tt)
        normalize(2)

        # lh = 1: standard lag-1 drafting; out_proj(0) fills the AV-free
        # window of the first score group.
        state["lh"], state["E"] = 1, [[None] * TT for _ in range(SG)]
        state["av"] = {}
        scores_group(0)
        out_proj(0)
        av_alloc(0)
        scores_group(1, av_plan=(0, True))
        av_alloc(1)
        scores_group(2, av_plan=(1, True))
        av_alloc(2)
        scores_group(3, av_plan=(2, True))
        out_proj(1)

    nc.compile()
    return nc


def _prep_inputs(x, conv_w, bn_gamma, bn_beta, bn_mean, bn_var, wq, wk, wv, wo):
    import ml_dtypes

    f32 = np.float32
    bf16 = ml_dtypes.bfloat16
    inv = (bn_gamma / np.sqrt(bn_var + BN_EPS)).astype(f32)
    w9 = (conv_w.reshape(C, 9) * inv[:, None]).astype(f32)
    bias = (bn_beta - bn_mean * inv).astype(f32).reshape(C, 1)
    wqT = np.ascontiguousarray(np.asarray(wq, f32).T).astype(bf16)
    wkT = np.ascontiguousarray(np.asarray(wk, f32).T).astype(bf16)
    wvT = np.ascontiguousarray(np.asarray(wv, f32).T).astype(bf16)
    woT = np.ascontiguousarray(np.asarray(wo, f32).T).astype(bf16)
    ones32 = np.ones((128, 32), f32)
    maps = []
    for b in range(B):
        maps.append(
            {
                "xt": np.ascontiguousarray(np.asarray(x[b], f32).T).astype(bf16),
                "w9": w9,
                "bias": bias,
                "wqT": wqT,
                "wkT": wkT,
                "wvT": wvT,
                "woT": woT,
                "ones32": ones32,
            }
        )
    return maps


def kernel(x, conv_w, bn_gamma, bn_beta, bn_mean, bn_var, wq, wk, wv, wo, h, w,
           **kw):
    assert int(h) == HH and int(w) == WW
    from concourse.bass_utils import run_bass_kernel_spmd

    if "nc" not in _CACHE:
        _CACHE["nc"] = _build()
    nc = _CACHE["nc"]
    maps = _prep_inputs(
        x, conv_w, bn_gamma, bn_beta, bn_mean, bn_var, wq, wk, wv, wo
    )
    res = run_bass_kernel_spmd(nc, maps, list(range(NCORES)))
    out = np.stack([res.results[b]["outT"].T for b in range(B)])
    return out.astype(np.float32)
